# revision 99
# baseline (speedup 1.0000x reference)
"""Trainium2 Bass kernel for nn_L1RegressionMoEActionHead.

Data-parallel over batch: 16 batch elements -> 2 per core x 8 cores.
Only the selected expert's weights are shipped (host-sliced).

v5: rope is folded into the score GEMM via the identity
  rope(q).rope(k) = [qc; qs] . [rope(k); -R rope(k)]
with qc = 8(q+b)cos, qs = 8(q+b)sin (R = interleaved rotate-half, R^2=-I).
Each q/k projection psum is drained by one fused scalar_tensor_tensor per
component ((psum + 2048 b) * table -> fp8), so the q side needs NO rotation
matmuls, combines, or Act copies; the k side needs two fp8-DoubleRow combine
matmuls ([I;R], [rhat;I]) plus one Act copy to build K4.  Scores contract
over 256 dims in fp8 DoubleRow (0.5 cyc/row).  Some heads route via an Act
q1-copy + Pool multiplies instead (QA_VIA/QT_VIA/K_VIA) to balance engines.

Other structure: o-proj accumulates the residual via a 512*I bf16 identity
matmul and LayerNorm stats read the psum directly (Newton rsqrt rescaled by
SRES); softmax denominator uses a [128,2,128] fp8 ones stationary (psum
lands broadcast); v/ffn biases enter as split-fp8 DoubleRow rank-2 matmuls;
FFN stays bf16 (fp8 costs ~3e-2 rel err, over the 2e-2 gate).

Schedule: R1 k/v (k-drain DVE, combine trails one head, v interleaved, last
vt groups at the tail); R2a q projections software-pipelined 4 heads ahead
of batch-0 attention; merged region: batch-1 attention + batch-0
o-proj/LN/FFN tiles; batch-1 phase-C tail in its own deep-pool region.
GPSIMD never touches PSUM (illegal on hw).  Output is bf16, widened on host.
"""

import math
import os

import numpy as np
import ml_dtypes

B = 16
T = 512
KA = 256
KT = 256
DIM = 1024
NH = 8
HD = 128
E = 8
EPS = 1e-5

NCORES = 8
BLOC = B // NCORES          # 2 batch elements per core
TOKQ = BLOC * T             # 1024 query tokens per core
TOKK = BLOC * KA            # 512 kv tokens per core (each of h_a / h_t)
NCT = DIM // 128            # 8 contraction tiles

BF16 = ml_dtypes.bfloat16
FP8 = ml_dtypes.float8_e4m3

SW = 256.0                  # weight fp8 scale (qkv projections)
SWO = 64.0                  # Wo fp8 scale
SX = 8.0                    # activation fp8 scale
SWF = 256.0                 # W_ffn fp8 scale
SZ = 1.0                    # LN-output scale (bf16 FFN)
SRES = 512.0                # o-proj psum scale (= SWO * 8, o8 is 8*true)

_CACHE = {}


def _rope_cos_sin(L):
    inv_freq = 1.0 / (10000.0 ** (np.arange(0, HD, 2, dtype=np.float32) / HD))
    freqs = np.arange(L, dtype=np.float32)[:, None] * inv_freq[None, :]
    emb = np.concatenate([freqs, freqs], axis=-1)   # (L, HD)
    return np.cos(emb), np.sin(emb)


def build_program():
    import concourse.bass as bass
    import concourse.mybir as mybir
    import concourse.tile as tile
    from concourse import bacc
    from contextlib import ExitStack

    f32 = mybir.dt.float32
    bf16 = mybir.dt.bfloat16
    fp8 = mybir.dt.float8e4
    AF = mybir.ActivationFunctionType
    ALU = mybir.AluOpType
    PM = mybir.MatmulPerfMode

    sc = 1.0 / math.sqrt(HD)

    # rope-drain route per head: heads listed here use Act(copy)+Pool(mults)
    # instead of two DVE scalar_tensor_tensor psum drains (engine balancing)
    K_VIA = frozenset()                 # k-side heads via Act+Pool
    QA_VIA = frozenset((0, 2, 4, 6))    # qa heads via Act+Pool
    QT_VIA = frozenset((1, 3, 5, 7))    # qt heads via Act+Pool
    QA_VIAD = frozenset()               # qa heads via Act+DVE (fast psum free)
    QT_VIAD = frozenset()               # qt heads via Act+DVE

    nc = bacc.Bacc("TRN2", target_bir_lowering=False, debug=False)

    def din(name, shape, dt):
        return nc.dram_tensor(name, list(shape), dt, kind="ExternalInput")

    xT = din("xT", (DIM, TOKQ), fp8)
    xnat = din("xnat", (TOKQ, DIM), bf16)       # x + b_o, residual stream
    haT = din("haT", (DIM, TOKK), fp8)
    htT = din("htT", (DIM, TOKK), fp8)

    wqaT = din("wqaT", (DIM, DIM), fp8)
    wqtT = din("wqtT", (DIM, DIM), fp8)
    wkaT = din("wkaT", (DIM, DIM), fp8)
    wktT = din("wktT", (DIM, DIM), fp8)
    wvaT = din("wvaT", (DIM, DIM), fp8)
    wvtT = din("wvtT", (DIM, DIM), fp8)
    woT = din("woT", (DIM, DIM), fp8)
    wfT = din("wfT", (DIM, DIM), bf16)

    biascols = din("biascols", (128, 4 * NH), f32)   # 2048*b (stt route)
    biascols8 = din("biascols8", (128, 4 * NH), f32)  # 8*b (act route)
    bva8 = din("bva8", (1, 2, DIM), fp8)             # split-fp8 2048*b_va rows
    bvt8 = din("bvt8", (1, 2, DIM), fp8)
    bf8 = din("bf8", (1, 2, DIM), fp8)               # split-fp8 2048*b_f rows
    gscale = din("gscale", (128, 1), f32)            # sc*g/64

    out_d = nc.dram_tensor("out", [TOKQ, DIM], bf16, kind="ExternalOutput")

    ASCALE = sc / (SX * SX)

    # ---------------- inline constants ----------------
    cos_q, sin_q = _rope_cos_sin(T)         # (T, HD)
    cos_k, sin_k = _rope_cos_sin(KA)        # (KA, HD)
    cq = np.ascontiguousarray(cos_q.T).astype(BF16)                   # (HD, T)
    sq = np.ascontiguousarray(sin_q.T).astype(BF16)
    ck = np.ascontiguousarray(np.tile(cos_k.T, (1, BLOC))).astype(BF16)
    sk = np.ascontiguousarray(np.tile(sin_k.T, (1, BLOC))).astype(BF16)

    # tables doubled so pair-wide (1024-elem) ops read them flat:
    # cosq2[0:1024] sinq2[1024:2048] cosk2[2048:3072] sink2[3072:4096]
    # rhatT[4096:4224] ident[4224:4352] ones_row[4352:4480]
    rhat = np.zeros((HD, HD), dtype=np.float32)
    _i = np.arange(0, HD, 2)
    rhat[_i, _i + 1] = -1.0
    rhat[_i + 1, _i] = 1.0
    # stt-route tables (psum * tab): tab = cos/ SW  (qc8 = 8*(q+b)*cos)
    # pool-route tables (q1 * tab):  tab = cos * SX
    s_stt = np.float32(1.0 / SW)
    s_q1 = np.float32(1.0)
    def b16(a):
        return np.ascontiguousarray(a.astype(np.float32)).astype(BF16)
    blob_bf = np.concatenate([
        b16(cq * s_stt), b16(cq * s_stt), b16(sq * s_stt), b16(sq * s_stt),
        b16(ck * s_stt), b16(ck * s_stt), b16(sk * s_stt), b16(sk * s_stt),
        b16(cq * s_q1), b16(cq * s_q1), b16(sq * s_q1), b16(sq * s_q1),
        b16(ck * s_q1), b16(ck * s_q1), b16(sk * s_q1), b16(sk * s_q1),
        np.eye(128, dtype=np.float32).astype(BF16),
        np.ones((128, 128), dtype=np.float32).astype(BF16),
        (SRES * np.eye(128, dtype=np.float32)).astype(BF16),
    ], axis=1)
    c_blob_bf = nc.inline_tensor(np.ascontiguousarray(blob_bf), "c_blob_bf")
    # K4 combine stationaries (fp8, [c, p] layout):
    #   top = kc + R ks   -> slot0 = I,      slot1 = rhat.T
    #   bot = ks - R kc   -> slot0 = -rhat.T, slot1 = I
    comb = np.zeros((128, 2, 2, 128), dtype=np.float32)
    comb[:, 0, 0, :] = np.eye(128)
    comb[:, 0, 1, :] = rhat.T
    comb[:, 1, 0, :] = -rhat.T
    comb[:, 1, 1, :] = np.eye(128)
    c_comb = nc.inline_tensor(np.ascontiguousarray(comb.astype(FP8)), "c_comb")

    with tile.TileContext(nc) as tc, ExitStack() as ctx:
        persist = ctx.enter_context(tc.tile_pool(name="persist", bufs=1))
        consts = ctx.enter_context(tc.tile_pool(name="consts", bufs=1))
        wpool = ctx.enter_context(tc.tile_pool(name="wpool", bufs=3))

        def cload(dram, shape, dt, tag):
            t = consts.tile(list(shape), dt, name=tag, tag=tag)
            nc.sync.dma_start(t[:], dram.ap())
            return t

        def load_w(wdram, dt=fp8):
            t = wpool.tile([128, NCT, DIM], dt, name="w", tag="w")
            ap = wdram.ap().rearrange("(a p) j -> p a j", p=128)
            nc.sync.dma_start(t[:, :, 0:DIM // 2], ap[:, :, 0:DIM // 2])
            nc.sync.dma_start(t[:, :, DIM // 2:], ap[:, :, DIM // 2:])
            return t

        # persistent activation tiles
        # Q2: [hd, head, (c|s), tok] fp8 = 8*(q+b) .* (cos | sin)
        # K4: [hd, head, (top|bot), tok] fp8 = 8*(rope(k) | -R rope(k))
        qa_all = persist.tile([HD, NH, 2, TOKQ], fp8, name="qa", tag="qa")
        qt_all = persist.tile([HD, NH, 2, TOKQ], fp8, name="qt", tag="qt")
        ka_all = persist.tile([HD, NH, 2, TOKK], fp8, name="ka", tag="ka")
        kt_all = persist.tile([HD, NH, 2, TOKK], fp8, name="kt", tag="kt")
        va_sb = [persist.tile([128, 2, DIM], fp8, name=f"va{b}", tag=f"va{b}") for b in range(BLOC)]
        vt_sb = [persist.tile([128, 2, DIM], fp8, name=f"vt{b}", tag=f"vt{b}") for b in range(BLOC)]
        o_sb = [persist.tile([HD, NH, T], fp8, name=f"o{b}", tag=f"o{b}") for b in range(BLOC)]
        wot = persist.tile([128, NCT, DIM], fp8, name="wo", tag="wo")
        wft = persist.tile([128, NCT, DIM], bf16, name="wf", tag="wf")
        xres = persist.tile([128, NCT, DIM], bf16, name="xres", tag="xres")

        # ---- DMA issue order (threads compute start against arrivals) ----
        # k/v weights go first; the constant blob is split so the k-side
        # tables (cols 2048:) land early while the q tables (cols 0:2048,
        # not needed until ~24us) load after the k/v weights; xT likewise.
        w_ka = wpool.tile([128, NCT, DIM], fp8, name="w", tag="w")
        _ka_ap = wkaT.ap().rearrange("(a p) j -> p a j", p=128)
        nc.sync.dma_start(w_ka[:, :, 0:DIM // 2], _ka_ap[:, :, 0:DIM // 2])
        sb_haT = consts.tile([128, NCT, TOKK], fp8, name="haT", tag="haT")
        _ha_ap = haT.ap().rearrange("(a p) t -> p a t", p=128)
        nc.sync.dma_start(sb_haT[:, 0:NCT // 2, :], _ha_ap[:, 0:NCT // 2, :])
        nc.sync.dma_start(sb_haT[:, NCT // 2:, :], _ha_ap[:, NCT // 2:, :])
        nc.sync.dma_start(w_ka[:, :, DIM // 2:], _ka_ap[:, :, DIM // 2:])
        sb_bias = cload(biascols, (128, 4 * NH), f32, "biasc")
        sb_bias8 = cload(biascols8, (128, 4 * NH), f32, "biasc8")
        sb_gs = cload(gscale, (128, 1), f32, "gs")
        sb_cb = consts.tile([128, blob_bf.shape[1]], bf16, name="cb", tag="cb")
        nc.sync.dma_start(sb_cb[:, 2048:4096], c_blob_bf.ap()[:, 2048:4096])
        nc.sync.dma_start(sb_cb[:, 8192:], c_blob_bf.ap()[:, 8192:])
        sb_comb = cload(c_comb, (128, 2, 2, 128), fp8, "comb")
        w_va = load_w(wvaT)
        sb_bva = cload(bva8, (1, 2, DIM), fp8, "bva")
        sb_htT = consts.tile([128, NCT, TOKK], fp8, name="htT", tag="htT")
        nc.sync.dma_start(sb_htT[:], htT.ap().rearrange("(a p) t -> p a t", p=128))

        sb_cosq2 = sb_cb[:, 0:1024]          # cos_q / SW
        sb_sinq2 = sb_cb[:, 1024:2048]
        sb_cosk2 = sb_cb[:, 2048:3072]
        sb_sink2 = sb_cb[:, 3072:4096]
        sb_cosq1 = sb_cb[:, 4096:5120]       # cos_q * SX
        sb_sinq1 = sb_cb[:, 5120:6144]
        sb_cosk1 = sb_cb[:, 6144:7168]
        sb_sink1 = sb_cb[:, 7168:8192]
        sb_ident = sb_cb[:, 8192:8320]
        sb_ones_row = sb_cb[0:1, 8320:8448]
        sb_identR = sb_cb[:, 8448:8576]
        # fused-drain table views: [cos|sin] as one [128, 2, *] AP
        sb_qcs = sb_cb[:, 0:2048].rearrange("p (s t) -> p s t", s=2)
        sb_qcs1 = sb_cb[:, 4096:6144].rearrange("p (s t) -> p s t", s=2)
        sb_kcs_tab = sb_cb[:, 2048:4096].rearrange("p (s t) -> p s t", s=2)[:, :, 0:512]
        sb_bqa = sb_bias[:, 0:NH]
        sb_bqt = sb_bias[:, NH:2 * NH]
        sb_bka = sb_bias[:, 2 * NH:3 * NH]
        sb_bkt = sb_bias[:, 3 * NH:4 * NH]
        sb8_bqa = sb_bias8[:, 0:NH]
        sb8_bqt = sb_bias8[:, NH:2 * NH]
        sb8_bka = sb_bias8[:, 2 * NH:3 * NH]
        sb8_bkt = sb_bias8[:, 3 * NH:4 * NH]

        ones8 = consts.tile([128, 2, 128], fp8, name="ones8", tag="ones8")
        nc.vector.memset(ones8[:], 1.0)

        # ============ Region 1: k/v projections ============
        with tc.tile_pool(name="kvps", bufs=2, space="PSUM") as kvps, \
             tc.tile_pool(name="k4ps", bufs=2, space="PSUM") as k4ps, \
             tc.tile_pool(name="vps", bufs=1, space="PSUM") as vps, \
             tc.tile_pool(name="ktmp", bufs=6) as ktmp:

            def k_drain(w, bias_sb, bias8_sb, src_sb, h, via):
                ps = kvps.tile([128, 512], f32, name="kps", tag="kps")
                for ct in range(NCT // 2):
                    nc.tensor.matmul(
                        ps[:],
                        w[:, 2 * ct:2 * ct + 2, h * 128:(h + 1) * 128],
                        src_sb[:, 2 * ct:2 * ct + 2, :],
                        start=(ct == 0), stop=(ct == NCT // 2 - 1),
                        perf_mode=PM.DoubleRow)
                kcs = ktmp.tile([128, 2, 512], fp8, name="kcs", tag="kcs")
                if h in via:
                    q1 = ktmp.tile([128, 512], bf16, name="kq1", tag="kq1")
                    nc.scalar.activation(q1[:], ps[:], AF.Identity,
                                         bias=bias8_sb[:, h:h + 1],
                                         scale=1.0 / SW)
                    nc.gpsimd.tensor_tensor(kcs[:, 0, :], q1[:],
                                            sb_cosk1[:, 0:512], op=ALU.mult)
                    nc.gpsimd.tensor_tensor(kcs[:, 1, :], q1[:],
                                            sb_sink1[:, 0:512], op=ALU.mult)
                else:
                    nc.vector.scalar_tensor_tensor(
                        kcs[:], ps[:].unsqueeze(1).broadcast_to([128, 2, 512]),
                        bias_sb[:, h:h + 1], sb_kcs_tab,
                        op0=ALU.add, op1=ALU.mult)
                return kcs

            def k_combine(out_all, h, kcs):
                # K4 = [kc + R ks ; ks - R kc]  (one DR matmul per slot)
                k4 = k4ps.tile([128, 2, 512], f32, name="k4", tag="k4")
                for sl_ in range(2):
                    nc.tensor.matmul(k4[:, sl_, :], sb_comb[:, sl_, :, :],
                                     kcs[:], start=True, stop=True,
                                     perf_mode=PM.DoubleRow)
                nc.scalar.activation(out_all[:, h], k4[:], AF.Identity)

            def v_group(w, src_sb, out_tiles, bias_bcast, idx,
                        copy_on_dve=False):
                b, kt_i = divmod(idx, 2)
                ksl = slice((b * 2 + kt_i) * 128, (b * 2 + kt_i + 1) * 128)
                ps = vps.tile([128, 2, 512], f32, name="vp", tag="vp")
                for jc in range(2):
                    for ct in range(NCT // 2):
                        nc.tensor.matmul(
                            ps[:, jc, :],
                            src_sb[:, 2 * ct:2 * ct + 2, ksl],
                            w[:, 2 * ct:2 * ct + 2, jc * 512:(jc + 1) * 512],
                            start=(ct == 0), stop=False,
                            perf_mode=PM.DoubleRow)
                    # + 2048*b_v as split-fp8 DoubleRow rank-2 matmul
                    nc.tensor.matmul(
                        ps[:, jc, :], ones8[0:1, :, :],
                        bias_bcast[:, :, jc * 512:(jc + 1) * 512],
                        start=False, stop=True,
                        perf_mode=PM.DoubleRow)
                # v8 = psum/SW   (fp8, = 8 * true v), one paired copy
                if copy_on_dve:
                    nc.vector.tensor_scalar(out_tiles[b][:, kt_i, :], ps[:],
                                            scalar1=1.0 / SW, scalar2=0.0,
                                            op0=ALU.mult, op1=ALU.add)
                else:
                    nc.scalar.activation(out_tiles[b][:, kt_i, :],
                                         ps[:], AF.Identity, scale=1.0 / SW)

            w_kt = load_w(wktT)
            w_vt = load_w(wvtT)
            sb_bvt = cload(bvt8, (1, 2, DIM), fp8, "bvt")
            # interleave k heads with v groups so DVE (k stt) and Act (v
            # copies) stay concurrently busy; combine trails drain by 1 head
            kq = []
            for h in range(NH + 1):
                if h < NH:
                    kq.append(k_drain(w_ka, sb_bka, sb8_bka, sb_haT, h, K_VIA))
                if h >= 1:
                    k_combine(ka_all, h - 1, kq[h - 1])
                if h % 2 == 1:
                    v_group(w_va, sb_haT, va_sb, sb_bva, h // 2)
            sb_xT = consts.tile([128, NCT, TOKQ], fp8, name="xT", tag="xT")
            nc.sync.dma_start(sb_xT[:, 0:NCT // 2, :],
                              xT.ap().rearrange("(a p) t -> p a t", p=128)[:, 0:NCT // 2, :])
            nc.sync.dma_start(sb_xT[:, NCT // 2:, :],
                              xT.ap().rearrange("(a p) t -> p a t", p=128)[:, NCT // 2:, :])
            w_qa = load_w(wqaT)
            nc.sync.dma_start(sb_cb[:, 0:2048], c_blob_bf.ap()[:, 0:2048])
            nc.sync.dma_start(sb_cb[:, 4096:6144], c_blob_bf.ap()[:, 4096:6144])
            kq = []
            for h in range(NH + 1):
                if h < NH:
                    kq.append(k_drain(w_kt, sb_bkt, sb8_bkt, sb_htT, h, K_VIA))
                if h == 0:
                    w_qt = load_w(wqtT)
                if h >= 1:
                    k_combine(kt_all, h - 1, kq[h - 1])
                if h % 2 == 1 and h // 2 < 2:
                    v_group(w_vt, sb_htT, vt_sb, sb_bvt, h // 2)
            v_group(w_vt, sb_htT, vt_sb, sb_bvt, 2, copy_on_dve=True)
            v_group(w_vt, sb_htT, vt_sb, sb_bvt, 3, copy_on_dve=True)
            sb_bf = cload(bf8, (1, 2, DIM), fp8, "bfr")

            # prefetch phase-C tensors
            nc.sync.dma_start(wot[:], woT.ap().rearrange("(a p) j -> p a j", p=128))
            nc.sync.dma_start(wft[:], wfT.ap().rearrange("(a p) j -> p a j", p=128))
            nc.sync.dma_start(xres[:], xnat.ap().rearrange("(a p) j -> p a j", p=128))



        def attn_scores(b, h, sp_pool, p_pool, singles=False):
            qsl = slice(b * T, (b + 1) * T)
            pa = p_pool.tile([128, 2, 512], fp8, name="pa", tag="pa")
            pt = p_pool.tile([128, 2, 512], fp8, name="pt", tag="pt")
            for kall, qall, ptile, esc in (
                    (ka_all, qa_all, pa, ASCALE),
                    (kt_all, qt_all, pt, sb_gs[:])):
                if singles:
                    for ci in range(2):
                        koff = b * KA + ci * 128
                        s = sp_pool.tile([128, 512], f32, name="ss", tag="ss")
                        nc.tensor.matmul(s[:],
                                         kall[:, h, :, koff:koff + 128],
                                         qall[:, h, :, qsl],
                                         start=True, stop=True,
                                         perf_mode=PM.DoubleRow)
                        nc.scalar.activation(ptile[:, ci, :], s[:], AF.Exp,
                                             scale=esc)
                else:
                    s = sp_pool.tile([128, 2, 512], f32, name="s", tag="s")
                    for ci in range(2):
                        koff = b * KA + ci * 128
                        nc.tensor.matmul(s[:, ci, :],
                                         kall[:, h, :, koff:koff + 128],
                                         qall[:, h, :, qsl],
                                         start=True, stop=True,
                                         perf_mode=PM.DoubleRow)
                    nc.scalar.activation(ptile[:], s[:], AF.Exp, scale=esc)
            return pa, pt

        def attn_out(b, h, pa, pt, d_pool, o_pool, r_pool):
            den = d_pool.tile([128, 512], f32, name="den", tag="den")
            for pi, ptile in enumerate((pa, pt)):
                nc.tensor.matmul(den[:], ones8[:], ptile[:],
                                 start=(pi == 0), stop=(pi == 1),
                                 skip_group_check=True,
                                 perf_mode=PM.DoubleRow)
            ov = o_pool.tile([128, 512], f32, name="ov", tag="ov")
            vtiles = (va_sb[b], vt_sb[b])
            for pi, ptile in enumerate((pa, pt)):
                nc.tensor.matmul(ov[:],
                                 vtiles[pi][:, :, h * 128:(h + 1) * 128],
                                 ptile[:], start=(pi == 0), stop=(pi == 1),
                                 skip_group_check=True,
                                 perf_mode=PM.DoubleRow)
            # o8 = ov * (1/den)  (= 8 * true attention output, fp8)
            recip = r_pool.tile([128, 512], f32, name="recip", tag="recip")
            nc.vector.reciprocal_approx_fast(recip[:], den[:])
            nc.vector.tensor_tensor(o_sb[b][:, h, :], ov[:], recip[:],
                                    op=ALU.mult)

        # ==== Region 2a: q projections + batch-0 attention ====
        with tc.tile_pool(name="qps", bufs=2, space="PSUM") as qps, \
             tc.tile_pool(name="sps", bufs=2, space="PSUM") as sps, \
             tc.tile_pool(name="dps", bufs=1, space="PSUM") as dps, \
             tc.tile_pool(name="ops", bufs=1, space="PSUM") as ops, \
             tc.tile_pool(name="qtmp", bufs=3) as qtmp, \
             tc.tile_pool(name="ptmp", bufs=6) as ptmp, \
             tc.tile_pool(name="rtmp", bufs=3) as rtmp:

            def q_head(w, bias_sb, bias8_sb, out_all, h, via=False,
                       viad=False):
                ps = qps.tile([128, 1024], f32, name="qpps", tag="qpps")
                for ci in range(2):
                    for ct in range(NCT // 2):
                        nc.tensor.matmul(
                            ps[:, ci * 512:(ci + 1) * 512],
                            w[:, 2 * ct:2 * ct + 2, h * 128:(h + 1) * 128],
                            sb_xT[:, 2 * ct:2 * ct + 2,
                                  ci * 512:(ci + 1) * 512],
                            start=(ct == 0), stop=(ct == NCT // 2 - 1),
                            perf_mode=PM.DoubleRow)
                if via or viad:
                    q1 = qtmp.tile([128, 1024], bf16, name="qq1", tag="qq1")
                    nc.scalar.activation(q1[:], ps[:], AF.Identity,
                                         bias=bias8_sb[:, h:h + 1],
                                         scale=1.0 / SW)
                    # one fused multiply: q1 (broadcast) * [cos|sin]
                    # (scalar_tensor_tensor is rejected on Pool by neuronxcc)
                    eng = nc.vector if viad else nc.gpsimd
                    eng.tensor_tensor(
                        out_all[:, h],
                        q1[:].unsqueeze(1).broadcast_to([128, 2, 1024]),
                        sb_qcs1, op=ALU.mult)
                else:
                    nc.vector.scalar_tensor_tensor(
                        out_all[:, h, 0, :], ps[:], bias_sb[:, h:h + 1],
                        sb_cosq2[:], op0=ALU.add, op1=ALU.mult)
                    nc.vector.scalar_tensor_tensor(
                        out_all[:, h, 1, :], ps[:], bias_sb[:, h:h + 1],
                        sb_sinq2[:], op0=ALU.add, op1=ALU.mult)

            # software-pipelined: attention for head h-2 issues after the
            # projections for head h, so exp/den/recip run 2 heads behind
            # the proj+drain front and every engine has independent work.
            for h in range(NH + 4):
                if h < NH:
                    q_head(w_qa, sb_bqa, sb8_bqa, qa_all, h,
                           via=(h in QA_VIA), viad=(h in QA_VIAD))
                    q_head(w_qt, sb_bqt, sb8_bqt, qt_all, h,
                           via=(h in QT_VIA), viad=(h in QT_VIAD))
                if h >= 4:
                    pa, pt = attn_scores(0, h - 4, sps, ptmp, singles=True)
                    attn_out(0, h - 4, pa, pt, dps, ops, rtmp)

        # ==== Attention (both batches) + o-proj/LN/FFN tiles ====
        # One elastic [128,512]-f32 psum pool serves scores/den/ov/o-proj/
        # transpose/ffn tiles so no sub-pipeline starves on a 1-buf pool.
        with tc.tile_pool(name="sps2", bufs=2, space="PSUM") as sps2, \
             tc.tile_pool(name="dps2", bufs=1, space="PSUM") as dps2, \
             tc.tile_pool(name="ops2", bufs=1, space="PSUM") as ops2, \
             tc.tile_pool(name="cps", bufs=2, space="PSUM") as cps, \
             tc.tile_pool(name="tps", bufs=1, space="PSUM") as tps, \
             tc.tile_pool(name="fps", bufs=1, space="PSUM") as fps, \
             tc.tile_pool(name="ptmp2", bufs=6) as ptmp2, \
             tc.tile_pool(name="rtmp2", bufs=3) as rtmp2, \
             tc.tile_pool(name="ctmp", bufs=4) as ctmp, \
             tc.tile_pool(name="zpool", bufs=4) as zpool, \
             tc.tile_pool(name="cres", bufs=3) as cres:

            NT4 = T // 128

            def phase_c_pre(b, t4, cps, ctmp, zpool, z_on_dve=False):
                # o-proj + residual accumulate in PSUM at scale SRES; LN stats
                # read the psum directly (no x2 SBUF roundtrip).  LN is scale-
                # invariant so only the Newton-rsqrt constants change.
                tt = b * NT4 + t4
                tsl = slice(t4 * 128, (t4 + 1) * 128)
                stats = ctmp.tile([128, 2, 6], f32, name="stats", tag="stats")
                pshalf = []
                for jc in range(2):
                    sl = slice(jc * 512, (jc + 1) * 512)
                    ps = cps.tile([128, 512], f32, name="op", tag="op")
                    for hp in range(NH // 2):
                        nc.tensor.matmul(
                            ps[:], o_sb[b][:, 2 * hp:2 * hp + 2, tsl],
                            wot[:, 2 * hp:2 * hp + 2, sl],
                            start=(hp == 0), stop=False,
                            perf_mode=PM.DoubleRow)
                    # + SRES * (x + b_o) via identity-stationary matmul
                    nc.tensor.matmul(ps[:], sb_identR[:],
                                     xres[:, tt, sl], start=False, stop=True)
                    nc.vector.bn_stats(stats[:, jc, :], ps[:])
                    pshalf.append(ps)
                mv = ctmp.tile([128, 2], f32, name="mv", tag="mv")
                nc.vector.bn_aggr(mv[:], stats[:])
                # rstd2 = SZ/sqrt(v_s), 2 Newton steps from seed 1/SRES
                # (v_s = SRES^2 * var_true, var_true in [0.8, 1.25])
                y1 = ctmp.tile([128, 1], f32, name="y1", tag="y1")
                nc.vector.tensor_scalar(
                    y1[:], mv[:, 1:2],
                    scalar1=-0.5 / SRES ** 3,
                    scalar2=(1.5 - 0.5 * EPS) / SRES,
                    op0=ALU.mult, op1=ALU.add)
                y1sq = ctmp.tile([128, 1], f32, name="y1sq", tag="y1sq")
                nc.vector.tensor_tensor(y1sq[:], y1[:], y1[:], op=ALU.mult)
                w_ = ctmp.tile([128, 1], f32, name="w_", tag="w_")
                nc.vector.tensor_tensor(w_[:], y1sq[:], mv[:, 1:2], op=ALU.mult)
                w2 = ctmp.tile([128, 1], f32, name="w2", tag="w2")
                nc.vector.tensor_scalar(w2[:], w_[:],
                                        scalar1=-0.5 * SZ, scalar2=1.5 * SZ,
                                        op0=ALU.mult, op1=ALU.add)
                rstd2 = ctmp.tile([128, 1], f32, name="rstd2", tag="rstd2")
                nc.vector.tensor_tensor(rstd2[:], w2[:], y1[:], op=ALU.mult)
                # z8 = SZ * (x2 - mu) * rstd  (fp8)
                z = zpool.tile([128, DIM], bf16, name="z", tag="z")
                zeng = nc.vector  # gpsimd cannot read PSUM on hw
                for jc in range(2):
                    zeng.tensor_scalar(z[:, jc * 512:(jc + 1) * 512],
                                       pshalf[jc][:],
                                       scalar1=mv[:, 0:1], scalar2=rstd2[:],
                                       op0=ALU.subtract, op1=ALU.mult)
                return z

            def phase_c_post(b, t4, z, tps, fps, ctmp, cres,
                             zt_on_act=False, relu_on_act=True):
                tt = b * NT4 + t4
                zT = []
                for half in range(2):
                    tp = tps.tile([128, 512], bf16, name="tp", tag="tp")
                    for q in range(4):
                        cb = half * 4 + q
                        nc.tensor.transpose(
                            tp[:, q * 128:(q + 1) * 128],
                            z[:, cb * 128:(cb + 1) * 128], sb_ident[:])
                    zt = ctmp.tile([128, 512], bf16, name=f"zT{half}", tag=f"zT{half}")
                    if zt_on_act:
                        nc.scalar.copy(zt[:], tp[:])
                    else:
                        nc.vector.tensor_copy(zt[:], tp[:])
                    zT.append(zt)
                row0 = tt * 128
                for jc in range(2):
                    sl = slice(jc * 512, (jc + 1) * 512)
                    fp = fps.tile([128, 512], f32, name="fp", tag="fp")
                    for ct in range(NCT):
                        nc.tensor.matmul(
                            fp[:],
                            zT[ct // 4][:, (ct % 4) * 128:(ct % 4 + 1) * 128],
                            wft[:, ct, sl],
                            start=(ct == 0), stop=False)
                    # + 2048*b_f as split-fp8 DoubleRow rank-2 matmul
                    nc.tensor.matmul(fp[:], ones8[0:1, :, :], sb_bf[:, :, sl],
                                     start=False, stop=True,
                                     perf_mode=PM.DoubleRow)
                    res = cres.tile([128, 512], bf16, name="res", tag="res")
                    if relu_on_act:
                        nc.scalar.activation(res[:], fp[:], AF.Relu)
                    else:
                        nc.vector.tensor_relu(res[:], fp[:])
                    nc.sync.dma_start(out_d.ap()[row0:row0 + 128, sl], res[:])

            z0 = []
            for h in range(NH):
                pa, pt = attn_scores(1, h, sps2, ptmp2, singles=True)
                attn_out(1, h, pa, pt, dps2, ops2, rtmp2)
                z0.append(phase_c_pre(0, h // 2, cps, ctmp, zpool)
                          if h % 2 == 0 else None)
                if h % 2 == 1:
                    phase_c_post(0, h // 2, z0[h - 1], tps, fps, ctmp, cres)
        # ==== batch-1 phase-C tail: deep pools (attention banks are free) ====
        with tc.tile_pool(name="cpsB", bufs=4, space="PSUM") as cpsB, \
             tc.tile_pool(name="tpsB", bufs=2, space="PSUM") as tpsB, \
             tc.tile_pool(name="fpsB", bufs=2, space="PSUM") as fpsB, \
             tc.tile_pool(name="ctmpB", bufs=4) as ctmpB, \
             tc.tile_pool(name="zpoolB", bufs=4) as zpoolB, \
             tc.tile_pool(name="cresB", bufs=3) as cresB:
            z1 = []
            for t4 in range(NT4 + 1):
                if t4 < NT4:
                    z1.append(phase_c_pre(1, t4, cpsB, ctmpB, zpoolB))
                if t4 >= 1:
                    phase_c_post(1, t4 - 1, z1[t4 - 1], tpsB, fpsB, ctmpB,
                                 cresB, zt_on_act=True, relu_on_act=False)

    nc.compile()
    return nc


def _prep_host(inputs):
    """Host-side preprocessing: expert select, scaling, transposes, sharding."""
    x = np.asarray(inputs["x"], dtype=np.float32)
    h_a = np.asarray(inputs["h_a"], dtype=np.float32)
    h_t = np.asarray(inputs["h_t"], dtype=np.float32)
    e = int(np.asarray(inputs["expert_idx"]))
    g = float(1.0 / (1.0 + math.exp(-float(np.asarray(inputs["gating_factor"])[e]))))
    sc = 1.0 / math.sqrt(HD)

    # rope-drain route per head: heads listed here use Act(copy)+Pool(mults)
    # instead of two DVE scalar_tensor_tensor psum drains (engine balancing)
    K_VIA = frozenset()                 # k-side heads via Act+Pool
    QA_VIA = frozenset((0, 2, 4, 6))    # qa heads via Act+Pool
    QT_VIA = frozenset((1, 3, 5, 7))    # qt heads via Act+Pool
    QA_VIAD = frozenset()               # qa heads via Act+DVE (fast psum free)
    QT_VIAD = frozenset()               # qt heads via Act+DVE

    def w8(w, scale):
        return np.ascontiguousarray(
            (np.asarray(w, dtype=np.float32)[e] * scale).T).astype(FP8)

    def bcol(bv, scale):
        return np.ascontiguousarray(
            (np.asarray(bv, dtype=np.float32)[e] * scale).reshape(NH, 128).T
        ).astype(np.float32)

    gamma = np.asarray(inputs["gamma"], dtype=np.float32)[e]
    beta = np.asarray(inputs["beta"], dtype=np.float32)[e]
    w_ffn = np.asarray(inputs["W_ffn"], dtype=np.float32)[e]
    b_ffn = np.asarray(inputs["b_ffn"], dtype=np.float32)[e]
    w_f_eff = w_ffn * gamma[None, :]
    b_f_eff = b_ffn + w_ffn @ beta
    b_o = np.asarray(inputs["b_o"], dtype=np.float32)[e]

    def split8(v):
        # row at psum scale -> (1, 2, n) fp8: r0 + r1 == v to fp8^2 accuracy
        r0 = v.astype(FP8)
        r1 = (v - r0.astype(np.float32)).astype(FP8)
        return np.ascontiguousarray(np.stack([r0, r1])[None])

    shared = {
        "wqaT": w8(inputs["W_qa"], SW),
        "wqtT": w8(inputs["W_qt"], SW),
        "wkaT": w8(inputs["W_ka"], SW),
        "wktT": w8(inputs["W_kt"], SW),
        "wvaT": w8(inputs["W_va"], SW),
        "wvtT": w8(inputs["W_vt"], SW),
        "woT": w8(inputs["W_o"], SWO),
        "wfT": np.ascontiguousarray(w_f_eff.T).astype(BF16),
        "biascols": np.ascontiguousarray(np.concatenate([
            bcol(inputs["b_qa"], SX * SW),
            bcol(inputs["b_qt"], SX * SW),
            bcol(inputs["b_ka"], SX * SW),
            bcol(inputs["b_kt"], SX * SW),
        ], axis=1)),
        "biascols8": np.ascontiguousarray(np.concatenate([
            bcol(inputs["b_qa"], SX),
            bcol(inputs["b_qt"], SX),
            bcol(inputs["b_ka"], SX),
            bcol(inputs["b_kt"], SX),
        ], axis=1)),
        "bva8": split8(np.asarray(inputs["b_va"], dtype=np.float32)[e]
                       * SX * SW),
        "bvt8": split8(np.asarray(inputs["b_vt"], dtype=np.float32)[e]
                       * SX * SW),
        "bf8": split8(b_f_eff),
        "gscale": np.full((128, 1), sc * g / (SX * SX), dtype=np.float32),
    }

    in_maps = []
    for c in range(NCORES):
        xc = x[c * BLOC:(c + 1) * BLOC].reshape(TOKQ, DIM)
        hac = h_a[c * BLOC:(c + 1) * BLOC].reshape(TOKK, DIM)
        htc = h_t[c * BLOC:(c + 1) * BLOC].reshape(TOKK, DIM)
        m = dict(shared)
        m["xT"] = np.ascontiguousarray(xc.T * SX).astype(FP8)
        m["xnat"] = np.ascontiguousarray(xc + b_o[None, :]).astype(BF16)
        m["haT"] = np.ascontiguousarray(hac.T * SX).astype(FP8)
        m["htT"] = np.ascontiguousarray(htc.T * SX).astype(FP8)
        in_maps.append(m)
    return in_maps


def run(inputs, trace=False):
    from concourse.bass_utils import run_bass_kernel_spmd

    if "nc" not in _CACHE:
        _CACHE["nc"] = build_program()
    nc = _CACHE["nc"]
    in_maps = _prep_host(inputs)
    res = run_bass_kernel_spmd(nc, in_maps, list(range(NCORES)), trace=trace)
    outs = [res.results[c]["out"].astype(np.float32).reshape(BLOC, T, DIM)
            for c in range(NCORES)]
    return np.concatenate(outs, axis=0), res


def kernel(**inputs) -> np.ndarray:
    out, _ = run(inputs, trace=False)
    return out



# revision 100
# speedup vs baseline: 1.0041x; 1.0041x over previous
"""Trainium2 Bass kernel for nn_L1RegressionMoEActionHead.

Data-parallel over batch: 16 batch elements -> 2 per core x 8 cores.
Only the selected expert's weights are shipped (host-sliced).

v5: rope is folded into the score GEMM via the identity
  rope(q).rope(k) = [qc; qs] . [rope(k); -R rope(k)]
with qc = 8(q+b)cos, qs = 8(q+b)sin (R = interleaved rotate-half, R^2=-I).
Each q/k projection psum is drained by one fused scalar_tensor_tensor per
component ((psum + 2048 b) * table -> fp8), so the q side needs NO rotation
matmuls, combines, or Act copies; the k side needs two fp8-DoubleRow combine
matmuls ([I;R], [rhat;I]) plus one Act copy to build K4.  Scores contract
over 256 dims in fp8 DoubleRow (0.5 cyc/row).  Some heads route via an Act
q1-copy + Pool multiplies instead (QA_VIA/QT_VIA/K_VIA) to balance engines.

Other structure: o-proj accumulates the residual via a 512*I bf16 identity
matmul and LayerNorm stats read the psum directly (Newton rsqrt rescaled by
SRES); softmax denominator uses a [128,2,128] fp8 ones stationary (psum
lands broadcast); v/ffn biases enter as split-fp8 DoubleRow rank-2 matmuls;
FFN stays bf16 (fp8 costs ~3e-2 rel err, over the 2e-2 gate).

Schedule: R1 k/v (k-drain DVE, combine trails one head, v interleaved, last
vt groups at the tail); R2a q projections software-pipelined 4 heads ahead
of batch-0 attention; merged region: batch-1 attention + batch-0
o-proj/LN/FFN tiles; batch-1 phase-C tail in its own deep-pool region.
GPSIMD never touches PSUM (illegal on hw).  Output is bf16, widened on host.
"""

import math
import os

import numpy as np
import ml_dtypes

B = 16
T = 512
KA = 256
KT = 256
DIM = 1024
NH = 8
HD = 128
E = 8
EPS = 1e-5

NCORES = 8
BLOC = B // NCORES          # 2 batch elements per core
TOKQ = BLOC * T             # 1024 query tokens per core
TOKK = BLOC * KA            # 512 kv tokens per core (each of h_a / h_t)
NCT = DIM // 128            # 8 contraction tiles

BF16 = ml_dtypes.bfloat16
FP8 = ml_dtypes.float8_e4m3

SW = 256.0                  # weight fp8 scale (qkv projections)
SWO = 64.0                  # Wo fp8 scale
SX = 8.0                    # activation fp8 scale
SWF = 256.0                 # W_ffn fp8 scale
SZ = 1.0                    # LN-output scale (bf16 FFN)
SRES = 512.0                # o-proj psum scale (= SWO * 8, o8 is 8*true)

_CACHE = {}


def _rope_cos_sin(L):
    inv_freq = 1.0 / (10000.0 ** (np.arange(0, HD, 2, dtype=np.float32) / HD))
    freqs = np.arange(L, dtype=np.float32)[:, None] * inv_freq[None, :]
    emb = np.concatenate([freqs, freqs], axis=-1)   # (L, HD)
    return np.cos(emb), np.sin(emb)


def build_program():
    import concourse.bass as bass
    import concourse.mybir as mybir
    import concourse.tile as tile
    from concourse import bacc
    from contextlib import ExitStack

    f32 = mybir.dt.float32
    bf16 = mybir.dt.bfloat16
    fp8 = mybir.dt.float8e4
    AF = mybir.ActivationFunctionType
    ALU = mybir.AluOpType
    PM = mybir.MatmulPerfMode

    sc = 1.0 / math.sqrt(HD)

    # rope-drain route per head: heads listed here use Act(copy)+Pool(mults)
    # instead of two DVE scalar_tensor_tensor psum drains (engine balancing)
    K_VIA = frozenset()                 # k-side heads via Act+Pool
    QA_VIA = frozenset((0, 2, 4, 6))    # qa heads via Act+Pool
    QT_VIA = frozenset((1, 3, 5, 7))    # qt heads via Act+Pool
    QA_VIAD = frozenset()               # qa heads via Act+DVE (fast psum free)
    QT_VIAD = frozenset()               # qt heads via Act+DVE

    nc = bacc.Bacc("TRN2", target_bir_lowering=False, debug=False)

    def din(name, shape, dt):
        return nc.dram_tensor(name, list(shape), dt, kind="ExternalInput")

    xT = din("xT", (DIM, TOKQ), fp8)
    xnat = din("xnat", (TOKQ, DIM), bf16)       # x + b_o, residual stream
    haT = din("haT", (DIM, TOKK), fp8)
    htT = din("htT", (DIM, TOKK), fp8)

    wqaT = din("wqaT", (DIM, DIM), fp8)
    wqtT = din("wqtT", (DIM, DIM), fp8)
    wkaT = din("wkaT", (DIM, DIM), fp8)
    wktT = din("wktT", (DIM, DIM), fp8)
    wvaT = din("wvaT", (DIM, DIM), fp8)
    wvtT = din("wvtT", (DIM, DIM), fp8)
    woT = din("woT", (DIM, DIM), fp8)
    wfT = din("wfT", (DIM, DIM), bf16)

    biascols = din("biascols", (128, 4 * NH), f32)   # 2048*b (stt route)
    biascols8 = din("biascols8", (128, 4 * NH), f32)  # 8*b (act route)
    bva8 = din("bva8", (1, 2, DIM), fp8)             # split-fp8 2048*b_va rows
    bvt8 = din("bvt8", (1, 2, DIM), fp8)
    bf8 = din("bf8", (1, 2, DIM), fp8)               # split-fp8 2048*b_f rows
    gscale = din("gscale", (128, 1), f32)            # sc*g/64

    out_d = nc.dram_tensor("out", [TOKQ, DIM], bf16, kind="ExternalOutput")

    ASCALE = sc / (SX * SX)

    # ---------------- inline constants ----------------
    cos_q, sin_q = _rope_cos_sin(T)         # (T, HD)
    cos_k, sin_k = _rope_cos_sin(KA)        # (KA, HD)
    cq = np.ascontiguousarray(cos_q.T).astype(BF16)                   # (HD, T)
    sq = np.ascontiguousarray(sin_q.T).astype(BF16)
    ck = np.ascontiguousarray(np.tile(cos_k.T, (1, BLOC))).astype(BF16)
    sk = np.ascontiguousarray(np.tile(sin_k.T, (1, BLOC))).astype(BF16)

    # tables doubled so pair-wide (1024-elem) ops read them flat:
    # cosq2[0:1024] sinq2[1024:2048] cosk2[2048:3072] sink2[3072:4096]
    # rhatT[4096:4224] ident[4224:4352] ones_row[4352:4480]
    rhat = np.zeros((HD, HD), dtype=np.float32)
    _i = np.arange(0, HD, 2)
    rhat[_i, _i + 1] = -1.0
    rhat[_i + 1, _i] = 1.0
    # stt-route tables (psum * tab): tab = cos/ SW  (qc8 = 8*(q+b)*cos)
    # pool-route tables (q1 * tab):  tab = cos * SX
    s_stt = np.float32(1.0 / SW)
    s_q1 = np.float32(1.0)
    def b16(a):
        return np.ascontiguousarray(a.astype(np.float32)).astype(BF16)
    blob_bf = np.concatenate([
        b16(cq * s_stt), b16(cq * s_stt), b16(sq * s_stt), b16(sq * s_stt),
        b16(ck * s_stt), b16(ck * s_stt), b16(sk * s_stt), b16(sk * s_stt),
        b16(cq * s_q1), b16(cq * s_q1), b16(sq * s_q1), b16(sq * s_q1),
        b16(ck * s_q1), b16(ck * s_q1), b16(sk * s_q1), b16(sk * s_q1),
        np.eye(128, dtype=np.float32).astype(BF16),
        np.ones((128, 128), dtype=np.float32).astype(BF16),
        (SRES * np.eye(128, dtype=np.float32)).astype(BF16),
    ], axis=1)
    c_blob_bf = nc.inline_tensor(np.ascontiguousarray(blob_bf), "c_blob_bf")
    # K4 combine stationaries (fp8, [c, p] layout):
    #   top = kc + R ks   -> slot0 = I,      slot1 = rhat.T
    #   bot = ks - R kc   -> slot0 = -rhat.T, slot1 = I
    comb = np.zeros((128, 2, 2, 128), dtype=np.float32)
    comb[:, 0, 0, :] = np.eye(128)
    comb[:, 0, 1, :] = rhat.T
    comb[:, 1, 0, :] = -rhat.T
    comb[:, 1, 1, :] = np.eye(128)
    c_comb = nc.inline_tensor(np.ascontiguousarray(comb.astype(FP8)), "c_comb")

    with tile.TileContext(nc) as tc, ExitStack() as ctx:
        persist = ctx.enter_context(tc.tile_pool(name="persist", bufs=1))
        consts = ctx.enter_context(tc.tile_pool(name="consts", bufs=1))
        wpool = ctx.enter_context(tc.tile_pool(name="wpool", bufs=3))

        def cload(dram, shape, dt, tag):
            t = consts.tile(list(shape), dt, name=tag, tag=tag)
            nc.sync.dma_start(t[:], dram.ap())
            return t

        def load_w(wdram, dt=fp8):
            t = wpool.tile([128, NCT, DIM], dt, name="w", tag="w")
            ap = wdram.ap().rearrange("(a p) j -> p a j", p=128)
            nc.sync.dma_start(t[:, :, 0:DIM // 2], ap[:, :, 0:DIM // 2])
            nc.sync.dma_start(t[:, :, DIM // 2:], ap[:, :, DIM // 2:])
            return t

        # persistent activation tiles
        # Q2: [hd, head, (c|s), tok] fp8 = 8*(q+b) .* (cos | sin)
        # K4: [hd, head, (top|bot), tok] fp8 = 8*(rope(k) | -R rope(k))
        qa_all = persist.tile([HD, NH, 2, TOKQ], fp8, name="qa", tag="qa")
        qt_all = persist.tile([HD, NH, 2, TOKQ], fp8, name="qt", tag="qt")
        ka_all = persist.tile([HD, NH, 2, TOKK], fp8, name="ka", tag="ka")
        kt_all = persist.tile([HD, NH, 2, TOKK], fp8, name="kt", tag="kt")
        va_sb = [persist.tile([128, 2, DIM], fp8, name=f"va{b}", tag=f"va{b}") for b in range(BLOC)]
        vt_sb = [persist.tile([128, 2, DIM], fp8, name=f"vt{b}", tag=f"vt{b}") for b in range(BLOC)]
        o_sb = [persist.tile([HD, NH, T], fp8, name=f"o{b}", tag=f"o{b}") for b in range(BLOC)]
        wot = persist.tile([128, NCT, DIM], fp8, name="wo", tag="wo")
        wft = persist.tile([128, NCT, DIM], bf16, name="wf", tag="wf")
        xres = persist.tile([128, NCT, DIM], bf16, name="xres", tag="xres")

        # ---- DMA issue order (threads compute start against arrivals) ----
        # k/v weights go first; the constant blob is split so the k-side
        # tables (cols 2048:) land early while the q tables (cols 0:2048,
        # not needed until ~24us) load after the k/v weights; xT likewise.
        w_ka = wpool.tile([128, NCT, DIM], fp8, name="w", tag="w")
        _ka_ap = wkaT.ap().rearrange("(a p) j -> p a j", p=128)
        nc.sync.dma_start(w_ka[:, :, 0:DIM // 2], _ka_ap[:, :, 0:DIM // 2])
        sb_haT = consts.tile([128, NCT, TOKK], fp8, name="haT", tag="haT")
        _ha_ap = haT.ap().rearrange("(a p) t -> p a t", p=128)
        nc.sync.dma_start(sb_haT[:, 0:NCT // 2, :], _ha_ap[:, 0:NCT // 2, :])
        nc.sync.dma_start(sb_haT[:, NCT // 2:, :], _ha_ap[:, NCT // 2:, :])
        nc.sync.dma_start(w_ka[:, :, DIM // 2:], _ka_ap[:, :, DIM // 2:])
        sb_bias = cload(biascols, (128, 4 * NH), f32, "biasc")
        sb_bias8 = cload(biascols8, (128, 4 * NH), f32, "biasc8")
        sb_gs = cload(gscale, (128, 1), f32, "gs")
        sb_cb = consts.tile([128, blob_bf.shape[1]], bf16, name="cb", tag="cb")
        nc.sync.dma_start(sb_cb[:, 2048:4096], c_blob_bf.ap()[:, 2048:4096])
        nc.sync.dma_start(sb_cb[:, 8192:], c_blob_bf.ap()[:, 8192:])
        sb_comb = cload(c_comb, (128, 2, 2, 128), fp8, "comb")
        w_va = load_w(wvaT)
        sb_bva = cload(bva8, (1, 2, DIM), fp8, "bva")
        sb_htT = consts.tile([128, NCT, TOKK], fp8, name="htT", tag="htT")
        nc.sync.dma_start(sb_htT[:], htT.ap().rearrange("(a p) t -> p a t", p=128))

        sb_cosq2 = sb_cb[:, 0:1024]          # cos_q / SW
        sb_sinq2 = sb_cb[:, 1024:2048]
        sb_cosk2 = sb_cb[:, 2048:3072]
        sb_sink2 = sb_cb[:, 3072:4096]
        sb_cosq1 = sb_cb[:, 4096:5120]       # cos_q * SX
        sb_sinq1 = sb_cb[:, 5120:6144]
        sb_cosk1 = sb_cb[:, 6144:7168]
        sb_sink1 = sb_cb[:, 7168:8192]
        sb_ident = sb_cb[:, 8192:8320]
        sb_ones_row = sb_cb[0:1, 8320:8448]
        sb_identR = sb_cb[:, 8448:8576]
        # fused-drain table views: [cos|sin] as one [128, 2, *] AP
        sb_qcs = sb_cb[:, 0:2048].rearrange("p (s t) -> p s t", s=2)
        sb_qcs1 = sb_cb[:, 4096:6144].rearrange("p (s t) -> p s t", s=2)
        sb_kcs_tab = sb_cb[:, 2048:4096].rearrange("p (s t) -> p s t", s=2)[:, :, 0:512]
        sb_bqa = sb_bias[:, 0:NH]
        sb_bqt = sb_bias[:, NH:2 * NH]
        sb_bka = sb_bias[:, 2 * NH:3 * NH]
        sb_bkt = sb_bias[:, 3 * NH:4 * NH]
        sb8_bqa = sb_bias8[:, 0:NH]
        sb8_bqt = sb_bias8[:, NH:2 * NH]
        sb8_bka = sb_bias8[:, 2 * NH:3 * NH]
        sb8_bkt = sb_bias8[:, 3 * NH:4 * NH]

        ones8 = consts.tile([128, 2, 128], fp8, name="ones8", tag="ones8")
        nc.vector.memset(ones8[:], 1.0)

        # ============ Region 1: k/v projections ============
        with tc.tile_pool(name="kvps", bufs=2, space="PSUM") as kvps, \
             tc.tile_pool(name="k4ps", bufs=2, space="PSUM") as k4ps, \
             tc.tile_pool(name="vps", bufs=1, space="PSUM") as vps, \
             tc.tile_pool(name="ktmp", bufs=6) as ktmp:

            def k_drain(w, bias_sb, bias8_sb, src_sb, h, via):
                ps = kvps.tile([128, 512], f32, name="kps", tag="kps")
                for ct in range(NCT // 2):
                    nc.tensor.matmul(
                        ps[:],
                        w[:, 2 * ct:2 * ct + 2, h * 128:(h + 1) * 128],
                        src_sb[:, 2 * ct:2 * ct + 2, :],
                        start=(ct == 0), stop=(ct == NCT // 2 - 1),
                        perf_mode=PM.DoubleRow)
                kcs = ktmp.tile([128, 2, 512], fp8, name="kcs", tag="kcs")
                if h in via:
                    q1 = ktmp.tile([128, 512], bf16, name="kq1", tag="kq1")
                    nc.scalar.activation(q1[:], ps[:], AF.Identity,
                                         bias=bias8_sb[:, h:h + 1],
                                         scale=1.0 / SW)
                    nc.gpsimd.tensor_tensor(kcs[:, 0, :], q1[:],
                                            sb_cosk1[:, 0:512], op=ALU.mult)
                    nc.gpsimd.tensor_tensor(kcs[:, 1, :], q1[:],
                                            sb_sink1[:, 0:512], op=ALU.mult)
                else:
                    nc.vector.scalar_tensor_tensor(
                        kcs[:], ps[:].unsqueeze(1).broadcast_to([128, 2, 512]),
                        bias_sb[:, h:h + 1], sb_kcs_tab,
                        op0=ALU.add, op1=ALU.mult)
                return kcs

            def k_combine(out_all, h, kcs):
                # K4 = [kc + R ks ; ks - R kc]  (one DR matmul per slot)
                k4 = k4ps.tile([128, 2, 512], f32, name="k4", tag="k4")
                for sl_ in range(2):
                    nc.tensor.matmul(k4[:, sl_, :], sb_comb[:, sl_, :, :],
                                     kcs[:], start=True, stop=True,
                                     perf_mode=PM.DoubleRow)
                nc.scalar.activation(out_all[:, h], k4[:], AF.Identity)

            def v_group(w, src_sb, out_tiles, bias_bcast, idx,
                        copy_on_dve=False):
                b, kt_i = divmod(idx, 2)
                ksl = slice((b * 2 + kt_i) * 128, (b * 2 + kt_i + 1) * 128)
                ps = vps.tile([128, 2, 512], f32, name="vp", tag="vp")
                for jc in range(2):
                    for ct in range(NCT // 2):
                        nc.tensor.matmul(
                            ps[:, jc, :],
                            src_sb[:, 2 * ct:2 * ct + 2, ksl],
                            w[:, 2 * ct:2 * ct + 2, jc * 512:(jc + 1) * 512],
                            start=(ct == 0), stop=False,
                            perf_mode=PM.DoubleRow)
                    # + 2048*b_v as split-fp8 DoubleRow rank-2 matmul
                    nc.tensor.matmul(
                        ps[:, jc, :], ones8[0:1, :, :],
                        bias_bcast[:, :, jc * 512:(jc + 1) * 512],
                        start=False, stop=True,
                        perf_mode=PM.DoubleRow)
                # v8 = psum/SW   (fp8, = 8 * true v), one paired copy
                if copy_on_dve:
                    nc.vector.tensor_scalar(out_tiles[b][:, kt_i, :], ps[:],
                                            scalar1=1.0 / SW, scalar2=0.0,
                                            op0=ALU.mult, op1=ALU.add)
                else:
                    nc.scalar.activation(out_tiles[b][:, kt_i, :],
                                         ps[:], AF.Identity, scale=1.0 / SW)

            w_kt = load_w(wktT)
            w_vt = load_w(wvtT)
            sb_bvt = cload(bvt8, (1, 2, DIM), fp8, "bvt")
            # interleave k heads with v groups so DVE (k stt) and Act (v
            # copies) stay concurrently busy; combine trails drain by 1 head
            kq = []
            for h in range(NH + 1):
                if h < NH:
                    kq.append(k_drain(w_ka, sb_bka, sb8_bka, sb_haT, h, K_VIA))
                if h >= 1:
                    k_combine(ka_all, h - 1, kq[h - 1])
                if h % 2 == 1:
                    v_group(w_va, sb_haT, va_sb, sb_bva, h // 2)
            sb_xT = consts.tile([128, NCT, TOKQ], fp8, name="xT", tag="xT")
            nc.sync.dma_start(sb_xT[:, 0:NCT // 2, :],
                              xT.ap().rearrange("(a p) t -> p a t", p=128)[:, 0:NCT // 2, :])
            nc.sync.dma_start(sb_xT[:, NCT // 2:, :],
                              xT.ap().rearrange("(a p) t -> p a t", p=128)[:, NCT // 2:, :])
            w_qa = load_w(wqaT)
            nc.sync.dma_start(sb_cb[:, 0:2048], c_blob_bf.ap()[:, 0:2048])
            nc.sync.dma_start(sb_cb[:, 4096:6144], c_blob_bf.ap()[:, 4096:6144])
            kq = []
            for h in range(NH + 1):
                if h < NH:
                    kq.append(k_drain(w_kt, sb_bkt, sb8_bkt, sb_htT, h, K_VIA))
                if h == 0:
                    w_qt = load_w(wqtT)
                if h >= 1:
                    k_combine(kt_all, h - 1, kq[h - 1])
                if h % 2 == 1 and h // 2 < 2:
                    v_group(w_vt, sb_htT, vt_sb, sb_bvt, h // 2)
            v_group(w_vt, sb_htT, vt_sb, sb_bvt, 2, copy_on_dve=True)
            v_group(w_vt, sb_htT, vt_sb, sb_bvt, 3, copy_on_dve=True)
            sb_bf = cload(bf8, (1, 2, DIM), fp8, "bfr")

            # prefetch phase-C tensors
            nc.sync.dma_start(wot[:], woT.ap().rearrange("(a p) j -> p a j", p=128))
            nc.sync.dma_start(wft[:], wfT.ap().rearrange("(a p) j -> p a j", p=128))
            nc.sync.dma_start(xres[:], xnat.ap().rearrange("(a p) j -> p a j", p=128))



        def attn_scores(b, h, sp_pool, p_pool, singles=False):
            qsl = slice(b * T, (b + 1) * T)
            pa = p_pool.tile([128, 2, 512], fp8, name="pa", tag="pa")
            pt = p_pool.tile([128, 2, 512], fp8, name="pt", tag="pt")
            for kall, qall, ptile, esc in (
                    (ka_all, qa_all, pa, ASCALE),
                    (kt_all, qt_all, pt, sb_gs[:])):
                if singles:
                    for ci in range(2):
                        koff = b * KA + ci * 128
                        s = sp_pool.tile([128, 512], f32, name="ss", tag="ss")
                        nc.tensor.matmul(s[:],
                                         kall[:, h, :, koff:koff + 128],
                                         qall[:, h, :, qsl],
                                         start=True, stop=True,
                                         perf_mode=PM.DoubleRow)
                        nc.scalar.activation(ptile[:, ci, :], s[:], AF.Exp,
                                             scale=esc)
                else:
                    s = sp_pool.tile([128, 2, 512], f32, name="s", tag="s")
                    for ci in range(2):
                        koff = b * KA + ci * 128
                        nc.tensor.matmul(s[:, ci, :],
                                         kall[:, h, :, koff:koff + 128],
                                         qall[:, h, :, qsl],
                                         start=True, stop=True,
                                         perf_mode=PM.DoubleRow)
                    nc.scalar.activation(ptile[:], s[:], AF.Exp, scale=esc)
            return pa, pt

        def attn_out(b, h, pa, pt, d_pool, o_pool, r_pool):
            den = d_pool.tile([128, 512], f32, name="den", tag="den")
            for pi, ptile in enumerate((pa, pt)):
                nc.tensor.matmul(den[:], ones8[:], ptile[:],
                                 start=(pi == 0), stop=(pi == 1),
                                 skip_group_check=True,
                                 perf_mode=PM.DoubleRow)
            ov = o_pool.tile([128, 512], f32, name="ov", tag="ov")
            vtiles = (va_sb[b], vt_sb[b])
            for pi, ptile in enumerate((pa, pt)):
                nc.tensor.matmul(ov[:],
                                 vtiles[pi][:, :, h * 128:(h + 1) * 128],
                                 ptile[:], start=(pi == 0), stop=(pi == 1),
                                 skip_group_check=True,
                                 perf_mode=PM.DoubleRow)
            # o8 = ov * (1/den)  (= 8 * true attention output, fp8)
            recip = r_pool.tile([128, 512], f32, name="recip", tag="recip")
            nc.vector.reciprocal_approx_fast(recip[:], den[:])
            nc.vector.tensor_tensor(o_sb[b][:, h, :], ov[:], recip[:],
                                    op=ALU.mult)

        # ==== Region 2a: q projections + batch-0 attention ====
        with tc.tile_pool(name="qps", bufs=2, space="PSUM") as qps, \
             tc.tile_pool(name="sps", bufs=2, space="PSUM") as sps, \
             tc.tile_pool(name="dps", bufs=1, space="PSUM") as dps, \
             tc.tile_pool(name="ops", bufs=1, space="PSUM") as ops, \
             tc.tile_pool(name="qtmp", bufs=3) as qtmp, \
             tc.tile_pool(name="ptmp", bufs=6) as ptmp, \
             tc.tile_pool(name="rtmp", bufs=3) as rtmp:

            def q_head(w, bias_sb, bias8_sb, out_all, h, via=False,
                       viad=False):
                ps = qps.tile([128, 1024], f32, name="qpps", tag="qpps")
                for ci in range(2):
                    for ct in range(NCT // 2):
                        nc.tensor.matmul(
                            ps[:, ci * 512:(ci + 1) * 512],
                            w[:, 2 * ct:2 * ct + 2, h * 128:(h + 1) * 128],
                            sb_xT[:, 2 * ct:2 * ct + 2,
                                  ci * 512:(ci + 1) * 512],
                            start=(ct == 0), stop=(ct == NCT // 2 - 1),
                            perf_mode=PM.DoubleRow)
                if via or viad:
                    q1 = qtmp.tile([128, 1024], bf16, name="qq1", tag="qq1")
                    nc.scalar.activation(q1[:], ps[:], AF.Identity,
                                         bias=bias8_sb[:, h:h + 1],
                                         scale=1.0 / SW)
                    # one fused multiply: q1 (broadcast) * [cos|sin]
                    # (scalar_tensor_tensor is rejected on Pool by neuronxcc)
                    eng = nc.vector if viad else nc.gpsimd
                    eng.tensor_tensor(
                        out_all[:, h],
                        q1[:].unsqueeze(1).broadcast_to([128, 2, 1024]),
                        sb_qcs1, op=ALU.mult)
                else:
                    nc.vector.scalar_tensor_tensor(
                        out_all[:, h, 0, :], ps[:], bias_sb[:, h:h + 1],
                        sb_cosq2[:], op0=ALU.add, op1=ALU.mult)
                    nc.vector.scalar_tensor_tensor(
                        out_all[:, h, 1, :], ps[:], bias_sb[:, h:h + 1],
                        sb_sinq2[:], op0=ALU.add, op1=ALU.mult)

            # software-pipelined: attention for head h-2 issues after the
            # projections for head h, so exp/den/recip run 2 heads behind
            # the proj+drain front and every engine has independent work.
            for h in range(NH + 4):
                if h < NH:
                    q_head(w_qa, sb_bqa, sb8_bqa, qa_all, h,
                           via=(h in QA_VIA), viad=(h in QA_VIAD))
                    q_head(w_qt, sb_bqt, sb8_bqt, qt_all, h,
                           via=(h in QT_VIA), viad=(h in QT_VIAD))
                if h >= 4:
                    pa, pt = attn_scores(0, h - 4, sps, ptmp, singles=True)
                    attn_out(0, h - 4, pa, pt, dps, ops, rtmp)

        # ==== Attention (both batches) + o-proj/LN/FFN tiles ====
        # One elastic [128,512]-f32 psum pool serves scores/den/ov/o-proj/
        # transpose/ffn tiles so no sub-pipeline starves on a 1-buf pool.
        with tc.tile_pool(name="sps2", bufs=2, space="PSUM") as sps2, \
             tc.tile_pool(name="dps2", bufs=1, space="PSUM") as dps2, \
             tc.tile_pool(name="ops2", bufs=1, space="PSUM") as ops2, \
             tc.tile_pool(name="cps", bufs=2, space="PSUM") as cps, \
             tc.tile_pool(name="tps", bufs=1, space="PSUM") as tps, \
             tc.tile_pool(name="fps", bufs=1, space="PSUM") as fps, \
             tc.tile_pool(name="ptmp2", bufs=6) as ptmp2, \
             tc.tile_pool(name="rtmp2", bufs=3) as rtmp2, \
             tc.tile_pool(name="ctmp", bufs=4) as ctmp, \
             tc.tile_pool(name="zpool", bufs=4) as zpool, \
             tc.tile_pool(name="cres", bufs=3) as cres:

            NT4 = T // 128

            def phase_c_pre(b, t4, cps, ctmp, zpool, z_on_dve=False):
                # o-proj + residual accumulate in PSUM at scale SRES; LN stats
                # read the psum directly (no x2 SBUF roundtrip).  LN is scale-
                # invariant so only the Newton-rsqrt constants change.
                tt = b * NT4 + t4
                tsl = slice(t4 * 128, (t4 + 1) * 128)
                stats = ctmp.tile([128, 2, 6], f32, name="stats", tag="stats")
                pshalf = []
                for jc in range(2):
                    sl = slice(jc * 512, (jc + 1) * 512)
                    ps = cps.tile([128, 512], f32, name="op", tag="op")
                    for hp in range(NH // 2):
                        nc.tensor.matmul(
                            ps[:], o_sb[b][:, 2 * hp:2 * hp + 2, tsl],
                            wot[:, 2 * hp:2 * hp + 2, sl],
                            start=(hp == 0), stop=False,
                            perf_mode=PM.DoubleRow)
                    # + SRES * (x + b_o) via identity-stationary matmul
                    nc.tensor.matmul(ps[:], sb_identR[:],
                                     xres[:, tt, sl], start=False, stop=True)
                    nc.vector.bn_stats(stats[:, jc, :], ps[:])
                    pshalf.append(ps)
                mv = ctmp.tile([128, 2], f32, name="mv", tag="mv")
                nc.vector.bn_aggr(mv[:], stats[:])
                # rstd2 = SZ/sqrt(v_s), 2 Newton steps from seed 1/SRES
                # (v_s = SRES^2 * var_true, var_true in [0.8, 1.25])
                y1 = ctmp.tile([128, 1], f32, name="y1", tag="y1")
                nc.vector.tensor_scalar(
                    y1[:], mv[:, 1:2],
                    scalar1=-0.5 / SRES ** 3,
                    scalar2=(1.5 - 0.5 * EPS) / SRES,
                    op0=ALU.mult, op1=ALU.add)
                y1sq = ctmp.tile([128, 1], f32, name="y1sq", tag="y1sq")
                nc.vector.tensor_tensor(y1sq[:], y1[:], y1[:], op=ALU.mult)
                w_ = ctmp.tile([128, 1], f32, name="w_", tag="w_")
                nc.vector.tensor_tensor(w_[:], y1sq[:], mv[:, 1:2], op=ALU.mult)
                w2 = ctmp.tile([128, 1], f32, name="w2", tag="w2")
                nc.vector.tensor_scalar(w2[:], w_[:],
                                        scalar1=-0.5 * SZ, scalar2=1.5 * SZ,
                                        op0=ALU.mult, op1=ALU.add)
                rstd2 = ctmp.tile([128, 1], f32, name="rstd2", tag="rstd2")
                nc.vector.tensor_tensor(rstd2[:], w2[:], y1[:], op=ALU.mult)
                # z8 = SZ * (x2 - mu) * rstd  (fp8)
                z = zpool.tile([128, DIM], bf16, name="z", tag="z")
                zeng = nc.vector  # gpsimd cannot read PSUM on hw
                for jc in range(2):
                    zeng.tensor_scalar(z[:, jc * 512:(jc + 1) * 512],
                                       pshalf[jc][:],
                                       scalar1=mv[:, 0:1], scalar2=rstd2[:],
                                       op0=ALU.subtract, op1=ALU.mult)
                return z

            def phase_c_post(b, t4, z, tps, fps, ctmp, cres,
                             zt_on_act=False, relu_on_act=True):
                tt = b * NT4 + t4
                zT = []
                for half in range(2):
                    tp = tps.tile([128, 512], bf16, name="tp", tag="tp")
                    for q in range(4):
                        cb = half * 4 + q
                        nc.tensor.transpose(
                            tp[:, q * 128:(q + 1) * 128],
                            z[:, cb * 128:(cb + 1) * 128], sb_ident[:])
                    zt = ctmp.tile([128, 512], bf16, name=f"zT{half}", tag=f"zT{half}")
                    if zt_on_act:
                        nc.scalar.copy(zt[:], tp[:])
                    else:
                        nc.vector.tensor_copy(zt[:], tp[:])
                    zT.append(zt)
                row0 = tt * 128
                for jc in range(2):
                    sl = slice(jc * 512, (jc + 1) * 512)
                    fp = fps.tile([128, 512], f32, name="fp", tag="fp")
                    for ct in range(NCT):
                        nc.tensor.matmul(
                            fp[:],
                            zT[ct // 4][:, (ct % 4) * 128:(ct % 4 + 1) * 128],
                            wft[:, ct, sl],
                            start=(ct == 0), stop=False)
                    # + 2048*b_f as split-fp8 DoubleRow rank-2 matmul
                    nc.tensor.matmul(fp[:], ones8[0:1, :, :], sb_bf[:, :, sl],
                                     start=False, stop=True,
                                     perf_mode=PM.DoubleRow)
                    res = cres.tile([128, 512], bf16, name="res", tag="res")
                    if relu_on_act:
                        nc.scalar.activation(res[:], fp[:], AF.Relu)
                    else:
                        nc.vector.tensor_relu(res[:], fp[:])
                    nc.sync.dma_start(out_d.ap()[row0:row0 + 128, sl], res[:])

            z0 = []
            for h in range(NH):
                pa, pt = attn_scores(1, h, sps2, ptmp2, singles=True)
                attn_out(1, h, pa, pt, dps2, ops2, rtmp2)
                z0.append(phase_c_pre(0, h // 2, cps, ctmp, zpool)
                          if h % 2 == 0 else None)
                if h % 2 == 1:
                    phase_c_post(0, h // 2, z0[h - 1], tps, fps, ctmp, cres)
        # ==== batch-1 phase-C tail: deep pools (attention banks are free) ====
        with tc.tile_pool(name="cpsB", bufs=3, space="PSUM") as cpsB, \
             tc.tile_pool(name="tpsB", bufs=3, space="PSUM") as tpsB, \
             tc.tile_pool(name="fpsB", bufs=2, space="PSUM") as fpsB, \
             tc.tile_pool(name="ctmpB", bufs=4) as ctmpB, \
             tc.tile_pool(name="zpoolB", bufs=4) as zpoolB, \
             tc.tile_pool(name="cresB", bufs=3) as cresB:
            z1 = []
            for t4 in range(NT4 + 1):
                if t4 < NT4:
                    z1.append(phase_c_pre(1, t4, cpsB, ctmpB, zpoolB))
                if t4 >= 1:
                    phase_c_post(1, t4 - 1, z1[t4 - 1], tpsB, fpsB, ctmpB,
                                 cresB, zt_on_act=True, relu_on_act=False)

    nc.compile()
    return nc


def _prep_host(inputs):
    """Host-side preprocessing: expert select, scaling, transposes, sharding."""
    x = np.asarray(inputs["x"], dtype=np.float32)
    h_a = np.asarray(inputs["h_a"], dtype=np.float32)
    h_t = np.asarray(inputs["h_t"], dtype=np.float32)
    e = int(np.asarray(inputs["expert_idx"]))
    g = float(1.0 / (1.0 + math.exp(-float(np.asarray(inputs["gating_factor"])[e]))))
    sc = 1.0 / math.sqrt(HD)

    # rope-drain route per head: heads listed here use Act(copy)+Pool(mults)
    # instead of two DVE scalar_tensor_tensor psum drains (engine balancing)
    K_VIA = frozenset()                 # k-side heads via Act+Pool
    QA_VIA = frozenset((0, 2, 4, 6))    # qa heads via Act+Pool
    QT_VIA = frozenset((1, 3, 5, 7))    # qt heads via Act+Pool
    QA_VIAD = frozenset()               # qa heads via Act+DVE (fast psum free)
    QT_VIAD = frozenset()               # qt heads via Act+DVE

    def w8(w, scale):
        return np.ascontiguousarray(
            (np.asarray(w, dtype=np.float32)[e] * scale).T).astype(FP8)

    def bcol(bv, scale):
        return np.ascontiguousarray(
            (np.asarray(bv, dtype=np.float32)[e] * scale).reshape(NH, 128).T
        ).astype(np.float32)

    gamma = np.asarray(inputs["gamma"], dtype=np.float32)[e]
    beta = np.asarray(inputs["beta"], dtype=np.float32)[e]
    w_ffn = np.asarray(inputs["W_ffn"], dtype=np.float32)[e]
    b_ffn = np.asarray(inputs["b_ffn"], dtype=np.float32)[e]
    w_f_eff = w_ffn * gamma[None, :]
    b_f_eff = b_ffn + w_ffn @ beta
    b_o = np.asarray(inputs["b_o"], dtype=np.float32)[e]

    def split8(v):
        # row at psum scale -> (1, 2, n) fp8: r0 + r1 == v to fp8^2 accuracy
        r0 = v.astype(FP8)
        r1 = (v - r0.astype(np.float32)).astype(FP8)
        return np.ascontiguousarray(np.stack([r0, r1])[None])

    shared = {
        "wqaT": w8(inputs["W_qa"], SW),
        "wqtT": w8(inputs["W_qt"], SW),
        "wkaT": w8(inputs["W_ka"], SW),
        "wktT": w8(inputs["W_kt"], SW),
        "wvaT": w8(inputs["W_va"], SW),
        "wvtT": w8(inputs["W_vt"], SW),
        "woT": w8(inputs["W_o"], SWO),
        "wfT": np.ascontiguousarray(w_f_eff.T).astype(BF16),
        "biascols": np.ascontiguousarray(np.concatenate([
            bcol(inputs["b_qa"], SX * SW),
            bcol(inputs["b_qt"], SX * SW),
            bcol(inputs["b_ka"], SX * SW),
            bcol(inputs["b_kt"], SX * SW),
        ], axis=1)),
        "biascols8": np.ascontiguousarray(np.concatenate([
            bcol(inputs["b_qa"], SX),
            bcol(inputs["b_qt"], SX),
            bcol(inputs["b_ka"], SX),
            bcol(inputs["b_kt"], SX),
        ], axis=1)),
        "bva8": split8(np.asarray(inputs["b_va"], dtype=np.float32)[e]
                       * SX * SW),
        "bvt8": split8(np.asarray(inputs["b_vt"], dtype=np.float32)[e]
                       * SX * SW),
        "bf8": split8(b_f_eff),
        "gscale": np.full((128, 1), sc * g / (SX * SX), dtype=np.float32),
    }

    in_maps = []
    for c in range(NCORES):
        xc = x[c * BLOC:(c + 1) * BLOC].reshape(TOKQ, DIM)
        hac = h_a[c * BLOC:(c + 1) * BLOC].reshape(TOKK, DIM)
        htc = h_t[c * BLOC:(c + 1) * BLOC].reshape(TOKK, DIM)
        m = dict(shared)
        m["xT"] = np.ascontiguousarray(xc.T * SX).astype(FP8)
        m["xnat"] = np.ascontiguousarray(xc + b_o[None, :]).astype(BF16)
        m["haT"] = np.ascontiguousarray(hac.T * SX).astype(FP8)
        m["htT"] = np.ascontiguousarray(htc.T * SX).astype(FP8)
        in_maps.append(m)
    return in_maps


def run(inputs, trace=False):
    from concourse.bass_utils import run_bass_kernel_spmd

    if "nc" not in _CACHE:
        _CACHE["nc"] = build_program()
    nc = _CACHE["nc"]
    in_maps = _prep_host(inputs)
    res = run_bass_kernel_spmd(nc, in_maps, list(range(NCORES)), trace=trace)
    outs = [res.results[c]["out"].astype(np.float32).reshape(BLOC, T, DIM)
            for c in range(NCORES)]
    return np.concatenate(outs, axis=0), res


def kernel(**inputs) -> np.ndarray:
    out, _ = run(inputs, trace=False)
    return out

